# revision 6
# baseline (speedup 1.0000x reference)
"""Trainium2 Bass kernel for nn_NetworkBasic (2-layer SLAYER SNN).

Pipeline per layer (per core, batch sharded 2/core across 8 cores):
  stage A (TensorE): temporal matmul  mid = dataT^T @ T   where
      T = c * P(srm-psp) @ D(2nd-difference), dataT is the 0/1 spike
      tensor pre-transposed to [(w2,t), (b,wp,h)] layout.  Layer 0's
      dataT comes pre-transposed from the HOST (free); layer 1's comes
      from PE transposes of the s1 spikes.
      T supplied as f16 hi+lo pair (2 accumulating matmuls).
  stage B (TensorE): spatial 3x3 conv as 3 h-contraction matmuls
      (banded [128,128] H_dw matrices from the runtime conv weights)
      with w-shifted PSUM accumulation -> What tensor.  Runs in fp32r
      (full-rate PE) by default, fp32 fallback via KERNEL_SPATIAL=f32.
  scan (VectorE): 2nd-order membrane recurrence, 2 ops/time-step:
      y_t     = (m[t] <= th) + 2d*m[t] + What[t+1]      (custom DVE op)
      m[t+1]  = -d^2 * m[t-1] + y_t                     (scalar_tensor_tensor)
  spikes: s = (m <= th) bulk threshold -> f16 (exact 0/1).

I/O: input is host-cast f16 + host-pre-transposed; output is f16 in
device-natural [h, (b,w,t)] layout, host reorders/casts to f32.

Membrane math: the refractory alpha kernel ref[k] = A*k*d^k is realized as
an IIR via scaled variables (scale c = 1/(A*d) < 0, which flips >= to <=).
The FIR truncation tail of the reference is ~1e-4 and is ignored.
"""

import os
import numpy as np

import concourse.bass as bass
import concourse.mybir as mybir
from concourse import bacc, bass_utils
from concourse.tile import TileContext
from concourse.masks import make_identity

F32 = mybir.dt.float32
F32R = mybir.dt.float32r
F16 = mybir.dt.float16
AO = mybir.AluOpType

# ---------------- problem constants (hardcoded) ----------------
B_FULL, H, W, T = 16, 128, 64, 64
N_CORES = 8
B_LOC = B_FULL // N_CORES          # 2
BW = B_LOC * W                     # 128 (b,w) columns per core
SP_FREE = BW * T                   # 8192 free elements ([128, 8192] tensors)
NPAIR = B_LOC * W // 2             # 64 transposed (b,w-pair) chunks
WG = 8                             # stage-B w-group size
NWG = W // WG                      # 8 w-groups

THETA = (30.0, 50.0)
TAU_SR = (1.0, 2.0)
TAU_REF = (1.0, 2.0)

SPATIAL = os.environ.get("KERNEL_SPATIAL", "fp32r")   # "fp32r" | "f32"


def _alpha_kernel(tau, mult, eps):
    vals = []
    for t in np.arange(0.0, float(T), 1.0):
        v = mult * t / tau * np.exp(1.0 - t / tau)
        if abs(v) < eps and t > tau:
            break
        vals.append(v)
    if len(vals) < 2:
        vals.append(0.0)
    return np.asarray(vals, np.float32)


SRM_K = [_alpha_kernel(TAU_SR[i], 1.0, 0.01) for i in range(2)]


def _layer_consts(layer):
    d = float(np.exp(-1.0 / TAU_REF[layer]))
    A = -2.0 * THETA[layer] * np.e / TAU_REF[layer]   # ref[k] = A*k*d^k
    c = 1.0 / (A * d)
    theta_hat = float(np.float32(c * THETA[layer]))
    return d, theta_hat


def _temporal_mat(layer):
    """[64,64] fp64 matrix:  what[t'] = sum_t data[t] * M[t, t']."""
    d, _ = _layer_consts(layer)
    A = -2.0 * THETA[layer] * np.e / TAU_REF[layer]
    c = 1.0 / (A * d)
    kern = SRM_K[layer].astype(np.float64)
    P = np.zeros((T, T))
    for t in range(T):
        for k in range(len(kern)):
            if t + k < T:
                P[t, t + k] = kern[k]
    D = np.zeros((T, T))
    for t in range(T):
        D[t, t] = 1.0
        if t + 1 < T:
            D[t, t + 1] = -2.0 * d
        if t + 2 < T:
            D[t, t + 2] = d * d
    return c * (P @ D)


def _hilo_f16(M):
    hi = M.astype(np.float16)
    lo = (M.astype(np.float32) - hi.astype(np.float32)).astype(np.float16)
    return hi, lo


def _hilo_f16_blockdiag(M):
    hi, lo = _hilo_f16(M)
    bhi = np.zeros((2 * T, 2 * T), np.float16)
    blo = np.zeros((2 * T, 2 * T), np.float16)
    for i in (0, 1):
        bhi[i * T:(i + 1) * T, i * T:(i + 1) * T] = hi
        blo[i * T:(i + 1) * T, i * T:(i + 1) * T] = lo
    return bhi, blo


def _h_mats(w):
    """w: [1,1,3,3] fp32 -> [3,128,128] fp32; Hm[dwi][h, hp] = w[h-hp+1, dwi]."""
    out = np.zeros((3, H, H), np.float32)
    for dwi in range(3):
        for dh in (-1, 0, 1):
            v = np.float32(w[0, 0, dh + 1, dwi])
            for hp in range(H):
                h = hp + dh
                if 0 <= h < H:
                    out[dwi, h, hp] = v
    return out


# ---------------- custom DVE op registration ----------------
_SNN_OP = None


def _register_snn_op():
    global _SNN_OP
    if _SNN_OP is not None:
        return _SNN_OP
    import concourse.dve_ops as dve_ops
    from concourse.dve_spec import Spec, Src0, Src1, C0, C1, lower
    from concourse.dve_uop import DveOpSpec

    name = "SNN_STEP_ANT"
    if name in dve_ops._SUB_OPCODE_FOR_NAME:
        _SNN_OP = next(op for op in dve_ops.OPS if op.name == name)
        return _SNN_OP

    # out = (s0 >= in0) + in0*s1 + in1
    body = (C0 >= Src0) + Src0 * C1 + Src1
    spec = Spec(
        body=body,
        reference=lambda in0, in1, s0, s1, imm2: (
            (np.float32(s0) >= in0).astype(np.float32)
            + in0 * np.float32(s1)
            + in1
        ).astype(np.float32),
    )
    row = 1 + len(dve_ops.OPS)
    shas = {}
    for ver in ("v3", "v4"):
        try:
            tmp = DveOpSpec(name=name, opcode=row, uops=lower(spec, ver=ver), rd1_en=True)
            shas[ver] = tmp.sha(ver)
        except Exception:
            pass
    op = dve_ops.DveOp(name, spec, subdim=False, uops_sha=shas)
    dve_ops.OPS.append(op)
    dve_ops._SUB_OPCODE_FOR_NAME[name] = row
    dve_ops.CUSTOM_DVE_SPECS[name] = spec
    _SNN_OP = op
    return op


# ---------------- bass kernel trace ----------------
def trace_kernel(nc, xt_d, t_d, h_d, out_d):
    """xt_d: [128, 8192] f16 dram (pre-transposed input);
    t_d: dict layer->(hi,lo) [2T,2T] f16 dram;
    h_d: dict layer->[3,128,128] f32 dram; out_d: [128, 8192] f16 dram."""
    snn_op = _register_snn_op()
    G = NPAIR // 4       # 16 stage-A groups of 4 chunks

    with TileContext(nc) as tc:
        with (
            tc.tile_pool(name="const", bufs=1) as cpool,
            tc.tile_pool(name="big", bufs=1) as bpool,
            tc.tile_pool(name="xtg", bufs=3) as xtpool,
            tc.tile_pool(name="scan", bufs=2) as ypool,
            tc.tile_pool(name="ptrans", bufs=2, space="PSUM") as pt_pool,
            tc.tile_pool(name="pa", bufs=2, space="PSUM") as pa_pool,
            tc.tile_pool(name="pb", bufs=2, space="PSUM") as pb_pool,
        ):
            # constants
            ident = cpool.tile([H, H], F16)
            make_identity(nc, ident)
            tmats = {}
            for layer in (0, 1):
                thi = cpool.tile([2 * T, 2 * T], F16, tag=f"thi{layer}")
                tlo = cpool.tile([2 * T, 2 * T], F16, tag=f"tlo{layer}")
                nc.sync.dma_start(out=thi, in_=t_d[layer][0].ap())
                nc.sync.dma_start(out=tlo, in_=t_d[layer][1].ap())
                tmats[layer] = (thi, tlo)
            hmats = {}
            HDT = F32R if SPATIAL == "fp32r" else F32
            for layer in (0, 1):
                hm = cpool.tile([H, 3 * H], HDT, tag=f"h{layer}")
                nc.sync.dma_start(
                    out=hm[:, :].rearrange("p (k n) -> p k n", k=3),
                    in_=h_d[layer].ap().rearrange("k p n -> p k n"),
                )
                hmats[layer] = hm

            # input: pre-transposed f16, DMA in 4 chunks for overlap with A
            dataT0 = bpool.tile([H, SP_FREE], F16, tag="dataT")
            for q in range(4):
                sl = slice(q * 2048, (q + 1) * 2048)
                nc.sync.dma_start(out=dataT0[:, sl], in_=xt_d.ap()[:, sl])

            dataT = dataT0
            for layer in (0, 1):
                d, theta_hat = _layer_consts(layer)
                thi, tlo = tmats[layer]
                hm = hmats[layer]

                # ---- stage A: block-diag temporal matmuls from dataT ----
                mid = bpool.tile([H, SP_FREE], HDT, tag="mid")
                scopeA = nc.enter_named_scope(f"stageA{layer}", False)
                for g in range(G):
                    pa = pa_pool.tile([H, 4 * H], F32, tag="pa")
                    for c2 in range(4):
                        chunk = g * 4 + c2
                        lhsT = dataT[:, chunk * H:(chunk + 1) * H]
                        nc.tensor.matmul(
                            pa[:, c2 * H:(c2 + 1) * H], lhsT, thi,
                            start=True, stop=False, skip_group_check=True,
                        )
                        nc.tensor.matmul(
                            pa[:, c2 * H:(c2 + 1) * H], lhsT, tlo,
                            start=False, stop=True, skip_group_check=True,
                        )
                    # alternate evacuation between Scalar and Vector engines
                    if g % 2 == 0:
                        nc.scalar.copy(mid[:, g * 512:(g + 1) * 512], pa)
                    else:
                        nc.vector.tensor_copy(mid[:, g * 512:(g + 1) * 512], pa)
                nc.leave_named_scope(f"stageA{layer}", scopeA[0], False)

                # ---- stage B: spatial conv, by (b, w-group), all t ----
                what = bpool.tile([H, SP_FREE], F32, tag="what")
                mview = mid[:, :].rearrange("p (b w t) -> p b w t", b=B_LOC, w=W)
                whatv = what[:, :].rearrange("p (b w t) -> p b w t", b=B_LOC, w=W)
                hm_mm, mv_mm = hm, mview
                scopeB = nc.enter_named_scope(f"stageB{layer}", False)
                nwg = 0
                for b in range(B_LOC):
                    for wg in range(NWG):
                        w0 = wg * WG
                        pb = pb_pool.tile([H, WG * T], F32, tag="pb")
                        pbv = pb[:, :].rearrange("p (w t) -> p w t", w=WG)
                        # center band
                        nc.tensor.matmul(
                            pbv[:, :, :], hm_mm[:, H:2 * H],
                            mv_mm[:, b, w0:w0 + WG, :],
                            start=True, stop=False, skip_group_check=True,
                        )
                        # left neighbor: out[w] += H_L @ mid[w-1]
                        if wg == 0:
                            nc.tensor.matmul(
                                pbv[:, 1:, :], hm_mm[:, 0:H],
                                mv_mm[:, b, 0:WG - 1, :],
                                start=False, stop=False, skip_group_check=True,
                            )
                        else:
                            nc.tensor.matmul(
                                pbv[:, :, :], hm_mm[:, 0:H],
                                mv_mm[:, b, w0 - 1:w0 + WG - 1, :],
                                start=False, stop=False, skip_group_check=True,
                            )
                        # right neighbor: out[w] += H_R @ mid[w+1]
                        if wg == NWG - 1:
                            nc.tensor.matmul(
                                pbv[:, :WG - 1, :], hm_mm[:, 2 * H:3 * H],
                                mv_mm[:, b, w0 + 1:w0 + WG, :],
                                start=False, stop=True, skip_group_check=True,
                            )
                        else:
                            nc.tensor.matmul(
                                pbv[:, :, :], hm_mm[:, 2 * H:3 * H],
                                mv_mm[:, b, w0 + 1:w0 + WG + 1, :],
                                start=False, stop=True, skip_group_check=True,
                            )
                        if nwg % 2 == 0:
                            nc.scalar.copy(whatv[:, b, w0:w0 + WG, :], pb[:, :].rearrange("p (w t) -> p w t", w=WG))
                        else:
                            nc.vector.tensor_copy(whatv[:, b, w0:w0 + WG, :], pb[:, :].rearrange("p (w t) -> p w t", w=WG))
                        nwg += 1
                nc.leave_named_scope(f"stageB{layer}", scopeB[0], False)

                # ---- scan ----
                scopeS = nc.enter_named_scope(f"scan{layer}", False)
                mh = bpool.tile([H, SP_FREE], F32, tag=f"mh{layer}")
                mh3 = mh[:, :].rearrange("p (bw t) -> p bw t", t=T)
                wS = what[:, :].rearrange("p (bw t) -> p bw t", t=T)

                def wslice(t):
                    return wS[:, :, t]

                nc.scalar.copy(mh3[:, :, 0], wslice(0))
                two_d = float(np.float32(2.0 * d))
                md2 = float(np.float32(-(d * d)))
                for t in range(T - 1):
                    if t == 0:
                        nc.vector._custom_dve(
                            snn_op, out=mh3[:, :, 1], in0=mh3[:, :, 0],
                            in1=wslice(1), s0=theta_hat, s1=two_d,
                        )
                    else:
                        y = ypool.tile([H, BW], F32, tag="y")
                        nc.vector._custom_dve(
                            snn_op, out=y, in0=mh3[:, :, t],
                            in1=wslice(t + 1), s0=theta_hat, s1=two_d,
                        )
                        nc.vector.scalar_tensor_tensor(
                            mh3[:, :, t + 1], mh3[:, :, t - 1], md2, y,
                            AO.mult, AO.add,
                        )
                nc.leave_named_scope(f"scan{layer}", scopeS[0], False)

                # ---- spikes ----
                if layer == 0:
                    s1 = bpool.tile([H, SP_FREE], F16, tag="dataT")
                    nc.vector.tensor_scalar(
                        s1, mh, theta_hat, None, AO.is_le,
                    )
                    # transpose s1 chunks on PE -> next layer's dataT
                    dataT = bpool.tile([H, SP_FREE], F16, tag="dataT2")
                    scopeT = nc.enter_named_scope("trans1", False)
                    for g in range(G):
                        ptr = pt_pool.tile([H, 4 * H], F16, tag="ptr")
                        for c2 in range(4):
                            chunk = g * 4 + c2
                            nc.tensor.transpose(
                                ptr[:, c2 * H:(c2 + 1) * H],
                                s1[:, chunk * H:(chunk + 1) * H],
                                ident,
                            )
                        if g % 2 == 0:
                            nc.scalar.copy(dataT[:, g * 512:(g + 1) * 512], ptr)
                        else:
                            nc.vector.tensor_copy(dataT[:, g * 512:(g + 1) * 512], ptr)
                    nc.leave_named_scope("trans1", scopeT[0], False)
                else:
                    s2 = bpool.tile([H, SP_FREE], F16, tag="s2")
                    nc.vector.tensor_scalar(
                        s2, mh, theta_hat, None, AO.is_le,
                    )
                    nc.sync.dma_start(out=out_d.ap(), in_=s2[:, :])
    return nc


_BUILT = {}


def _build():
    global _BUILT
    key = (SPATIAL,)
    if key in _BUILT:
        return _BUILT[key]
    nc = bacc.Bacc("TRN2", debug=False)
    xt_d = nc.dram_tensor("xt", [H, SP_FREE], F16, kind="ExternalInput")
    t_d, h_d = {}, {}
    for layer in (0, 1):
        t_d[layer] = (
            nc.dram_tensor(f"t{layer}hi", [2 * T, 2 * T], F16, kind="ExternalInput"),
            nc.dram_tensor(f"t{layer}lo", [2 * T, 2 * T], F16, kind="ExternalInput"),
        )
        h_d[layer] = nc.dram_tensor(
            f"h{layer}", [3, H, H],
            F32R if SPATIAL == "fp32r" else F32, kind="ExternalInput")
    out_d = nc.dram_tensor("out", [H, SP_FREE], F16, kind="ExternalOutput")
    trace_kernel(nc, xt_d, t_d, h_d, out_d)
    nc.compile()
    _BUILT[key] = nc
    return nc


def _host_inputs(conv1_w, conv2_w):
    """Common (replicated) input tensors, computed on host."""
    ins = {}
    for layer, w in ((0, conv1_w), (1, conv2_w)):
        hi, lo = _hilo_f16_blockdiag(_temporal_mat(layer))
        ins[f"t{layer}hi"] = hi
        ins[f"t{layer}lo"] = lo
        ins[f"h{layer}"] = _h_mats(np.asarray(w, np.float32))
    return ins


def make_in_maps(spikeInput, conv1_w, conv2_w):
    x = np.asarray(spikeInput, np.float32).reshape(B_FULL, H, W, T)
    x16 = x.astype(np.float16)                      # exact: values are 0/1
    common = _host_inputs(conv1_w, conv2_w)
    in_maps = []
    for c in range(N_CORES):
        xc = x16[c * B_LOC:(c + 1) * B_LOC]         # [b, h, w, t]
        xc = xc.reshape(B_LOC, H, W // 2, 2, T)     # b h wp w2 t
        xt = np.ascontiguousarray(xc.transpose(3, 4, 0, 2, 1))  # w2 t b wp h
        m = dict(common)
        m["xt"] = xt.reshape(H, SP_FREE)
        in_maps.append(m)
    return in_maps


def kernel(spikeInput, conv1_w, conv2_w):
    nc = _build()
    in_maps = make_in_maps(spikeInput, conv1_w, conv2_w)
    res = bass_utils.run_bass_kernel_spmd(nc, in_maps, core_ids=list(range(N_CORES)))
    outs = []
    for r in res.results:
        o = r["out"].reshape(H, B_LOC, W, T)        # h b w t
        outs.append(o.transpose(1, 0, 2, 3))        # b h w t
    return np.concatenate(outs, axis=0).astype(np.float32)
